# revision 38
# baseline (speedup 1.0000x reference)
"""DeltaNet-style gated linear attention forward on 8 Trainium2 NeuronCores.

Sharding: core c = (batch b = c//4, sequence quarter r = c%4). Each core
projects q/k/v/gate for its 512 rows (all 16 heads), runs chunked linear
attention in quarter-local unscaled coordinates, exchanges per-quarter state
summaries via a small AllGather, then applies the inter-quarter state, output
projection, residual and LayerNorm for its own rows.

Math (per batch, head), matching the reference scan: with
b_i = prod_{j<=i} f_j (cumprod from QUARTER start), q~_i = phi_i * b_i,
k~_j = phi_j / b_j, the output row i is
  out_i = q~_i (S_start + U_i) / max(q~_i . (m_start + mU_i), eps)
where U_i = sum_{j<=i} k~_j v_j^T accumulates unscaled across the quarter and
(S_start, m_start) is the true state entering the quarter, combined from the
peer quarters' summaries (A_q, A_q * U_q) after an AllGather.

Schedule: the gate chain runs transposed ([H, QT]) with a DVE prefix scan;
k/v projections + ΔU summaries for all 4 chunks complete first and the
AllGather fires early; the whole q-side (projection, rope/phi, intra-chunk
attention, q.U_prev) overlaps the collective; q.S_start closes into the same
accumulators after the gather. PE work is emitted to stay continuously busy
(p-state ramp), elementwise work is fused via scalar_tensor_tensor and spread
across DVE/Pool/ACT.
"""

import contextlib

import numpy as np
import ml_dtypes

import bass_rust
import concourse.bass as bass
import concourse.mybir as mybir
import concourse.tile as tile
from concourse.bass_utils import run_bass_kernel_spmd

dt = mybir.dt
AF = mybir.ActivationFunctionType
ALU = mybir.AluOpType

B, T, C, H, D = 2, 2048, 1024, 16, 64
NCORE = 8
QT = T // 4          # rows per core
L = 128              # chunk length
NCH = QT // L        # chunks per core
KT = C // 128        # contraction tiles
SE = D + 1           # state row width (S | m)
ROPE_BASE = 10000.0
EPS = 1e-6
LN_EPS = 1e-5
G_CLAMP = -30.0      # cumsum floor (defensive; inert for real data)
SPLIT_WAITS = True


# ---------------------------------------------------------------- walrus shim
def _split_multi_waits(nc):
    ctr = 0
    for fn in nc.m.functions:
        for bb in fn.blocks:
            out = []
            for ins in bb.instructions:
                si = ins.sync_info
                if si is not None and si.on_wait and len(si.on_wait) > 1:
                    waits = list(si.on_wait)
                    for w in waits[:-1]:
                        ctr += 1
                        nop = mybir.InstNoOp(name=f"WS-{ctr}", ins=[], outs=[])
                        nop.engine = ins.engine
                        nop.sync_info = bass_rust.SyncInfo(on_wait=[w], on_update=[])
                        nop.debug = ins.debug
                        out.append(nop)
                    si.on_wait = [waits[-1]]
                out.append(ins)
            bb.instructions[:] = out
    return ctr


def _register_const(nc, value, dtype=dt.float32):
    t = nc.alloc_sbuf_tensor(f"uconst-{dtype.name}-{value}", [128, 1], dtype)
    nc.gpsimd.memset(t.ap(), value)
    nc.const_aps.aps[(dtype, value)] = t.ap()


def _enable_ldw_opt():
    try:
        from concourse.compiler_utils import get_compiler_flags, set_compiler_flags
        flags = get_compiler_flags()
        new = [f.replace("--enable-ldw-opt=false", "--enable-ldw-opt=true")
               for f in flags]
        if new != flags:
            set_compiler_flags(new)
    except Exception:
        pass


# patch the hardcoded --enable-ldw-opt=false in the walrus invocation
import concourse.bass_utils as _bu
if not getattr(_bu, "_ldw_patched", False):
    _orig_run_command = _bu.run_command

    def _run_command_ldw(cmd, *a, **kw):
        # ldw-opt breaks on the framework's wait-motion Ldweights split;
        # leave the backend flag as-is.
        return _orig_run_command(cmd, *a, **kw)

    _bu.run_command = _run_command_ldw
    _bu._ldw_patched = True


# ------------------------------------------------------------------- builder
def build(has_mask=False, has_ln=False):
    _enable_ldw_opt()
    nc = bass.Bass(target_bir_lowering=False, debug=False)
    _register_const(nc, float(LN_EPS))
    nc.all_engine_barrier()

    f32 = dt.float32
    bf16 = dt.bfloat16
    P = {}

    def param(name, shape, dtype=f32, out=False):
        P[name] = nc.declare_dram_parameter(name, list(shape), dtype, isOutput=out)
        return P[name]

    param("xTb", (128, KT, QT), bf16)           # x rows^T (bf16)
    param("wqkv", (128, KT, 3 * C), bf16)       # [Wq.T|Wk.T|Wv.T] k-tiled
    param("wgT", (128, KT, H), bf16)            # Wg.T k-tiled (gate lhsT)
    param("wo", (128, KT, C), bf16)             # Wo.T k-tiled
    param("xrows", (QT, C))                     # residual rows (+bo folded in)
    param("ropec", (128, NCH, 128), bf16)             # [cos|cos|-sin|+sin] per chunk
    param("triu", (128, 128), bf16)             # j<=i ones (causal mask)
    param("eye", (128, 128), bf16)              # PE transpose identity (bf16)
    param("eyef", (16, 16))                     # PE transpose identity (fp32)
    param("onesrow", (1, 128))
    param("bgcol", (16, 1))                     # gate bias per head (column)
    param("sel", (64, 4))                       # quarter-combine select (q < r)
    param("isel", (64, 4))                      # 1 - sel
    if has_mask:
        param("mkc", (128, NCH))                # mask, chunk-major columns
        param("mkT", (16, QT))                  # mask broadcast over 16 parts
    if has_ln:
        param("lnw", (128, C))
        param("lnb", (128, C))
    param("y", (QT, C), out=True)

    one_b = nc.const_aps.aps[(bf16, 1.0)]

    with tile.TileContext(nc) as tc, contextlib.ExitStack() as outer:
        keep = outer.enter_context(tc.tile_pool(name="keep", bufs=1))
        dram = outer.enter_context(tc.tile_pool(name="dram", bufs=1, space="DRAM"))

        # ---- constants to SBUF
        triu_sb = keep.tile([128, 128], bf16, name="triu_sb")
        eye_sb = keep.tile([128, 128], bf16, name="eye_sb")
        eyef_sb = keep.tile([16, 16], f32, name="eyef_sb")
        ones_row = keep.tile([1, 128], f32, name="ones_row")
        bg_sb = keep.tile([16, 1], f32, name="bg_sb")
        rope_sb = keep.tile([128, NCH, 128], bf16, name="rope_sb")
        sel_sb = keep.tile([64, 4], f32, name="sel_sb")
        isel_sb = keep.tile([64, 4], f32, name="isel_sb")
        for t_, p_ in ((triu_sb, "triu"), (eye_sb, "eye"), (eyef_sb, "eyef"),
                       (ones_row, "onesrow"), (bg_sb, "bgcol"),
                       (rope_sb, "ropec"), (sel_sb, "sel"), (isel_sb, "isel")):
            nc.sync.dma_start(t_[:], P[p_][:])
        if has_mask:
            mkc_sb = keep.tile([128, NCH], f32, name="mkc_sb")
            mkT_sb = keep.tile([16, QT], f32, name="mkT_sb")
            nc.sync.dma_start(mkc_sb[:], P["mkc"][:])
            nc.sync.dma_start(mkT_sb[:], P["mkT"][:])

        # ---- big inputs (emit early; consumed as they land)
        wgT_sb = keep.tile([128, KT, H], bf16, name="wgT_sb")
        nc.sync.dma_start(wgT_sb[:], P["wgT"][:])
        xtb_sb = keep.tile([128, KT, QT], bf16, name="xtb_sb")
        for kt in range(KT):
            nc.sync.dma_start(xtb_sb[:, kt, :], P["xTb"][:, kt, :])
        wq_sb = keep.tile([128, KT, C], bf16, name="wq_sb")
        wo_sb = keep.tile([128, KT, C], bf16, name="wo_sb")
        if has_ln:
            lnw_sb = keep.tile([128, C], f32, name="lnw_sb")
            lnb_sb = keep.tile([128, C], f32, name="lnb_sb")
            nc.sync.dma_start(lnw_sb[:], P["lnw"][:])
            nc.sync.dma_start(lnb_sb[:], P["lnb"][:])

        # persistent per-chunk tiles
        ktp = outer.enter_context(tc.tile_pool(name="ktp", bufs=NCH))
        qtp = outer.enter_context(tc.tile_pool(name="qtp", bufs=NCH))
        vxp = outer.enter_context(tc.tile_pool(name="vxp", bufs=NCH))
        tnp = outer.enter_context(tc.tile_pool(name="tnp", bufs=NCH))
        usp = outer.enter_context(tc.tile_pool(name="usp", bufs=NCH))
        wrk = outer.enter_context(tc.tile_pool(name="wrk", bufs=2))
        sm = outer.enter_context(tc.tile_pool(name="sm", bufs=2))

        kt_tiles, qt_tiles, vx_tiles, tn_tiles, u_tiles = [], [], [], [], []
        sacb_tiles = []

        # gate result tiles (whole quarter)
        b_rm = keep.tile([128, NCH, H], bf16, name="b_rm")   # b
        bi_rm = keep.tile([128, NCH, H], bf16, name="bi_rm")  # 1/b
        arun = keep.tile([64, H], f32, name="arun")         # A broadcast rows

        # ---------------------------------------------------- shared helpers
        def rope_phi(ch, psum, which):
            """psum [128, C] fp32 -> row-major scaled phi tile bf16."""
            pv = psum.rearrange("p (h d) -> p h d", h=H)
            cs = rope_sb[:, ch, 0:64]
            sna = rope_sb[:, ch, 64:96]
            snb = rope_sb[:, ch, 96:128]
            rr = wrk.tile([128, H, D], bf16, name="rr", tag="rr")
            tmp = wrk.tile([128, H, D], bf16, name="tmp", tag="tmp", bufs=1)
            nc.vector.tensor_mul(
                rr[:], pv[:], cs[:, None, :].to_broadcast([128, H, D]))
            nc.vector.tensor_mul(
                tmp[:, :, 0:32], pv[:, :, 32:64],
                sna[:, None, :].to_broadcast([128, H, 32]))
            nc.vector.tensor_mul(
                tmp[:, :, 32:64], pv[:, :, 0:32],
                snb[:, None, :].to_broadcast([128, H, 32]))
            rrf = rr.rearrange("p h d -> p (h d)")
            nc.gpsimd.tensor_add(rrf, rrf, tmp.rearrange("p h d -> p (h d)"))
            # phi = exp(min(r,0)) + relu(r); then scale by b (or 1/b)
            mn = wrk.tile([128, H, D], bf16, name="mn", tag="mn")
            mnf = mn.rearrange("p h d -> p (h d)")
            nc.vector.tensor_scalar_min(mnf, rrf, 0.0)
            nc.scalar.activation(mnf, mnf, AF.Exp)
            sk = wrk.tile([128, H, D], bf16, name="sk", tag="tmp", bufs=1)
            nc.vector.scalar_tensor_tensor(
                sk[:], rr[:], 0.0, mn[:], op0=ALU.max, op1=ALU.add)
            scale = b_rm if which == "q" else bi_rm
            out = wrk.tile([128, H, D], bf16, name="ph", tag="mn")
            nc.gpsimd.tensor_mul(
                out[:], sk[:],
                scale[:, ch, :, None].to_broadcast([128, H, D]))
            return out

        def transpose_to(dst, src, pool):
            """src row-major [128, H, D] bf16 -> dst [64, H, 128] via PE."""
            sf = src.rearrange("p h d -> p (h d)")
            for g in range(4):
                tps = pool.tile([64, 4, 128], bf16, name="tps", tag="tps")
                for j in range(4):
                    h = g * 4 + j
                    nc.tensor.matmul(tps[:, j, :], sf[:, bass.ts(h, 64)],
                                     eye_sb[:], is_transpose=True,
                                     start=(j == 0), stop=(j == 3))
                sl = slice(g * 4, (g + 1) * 4)
                if g % 2 == 0:
                    nc.vector.tensor_copy(dst[:, sl, :], tps[:])
                else:
                    nc.scalar.copy(dst[:, sl, :], tps[:])

        # ================================================= scope 1: gate + kv
        with contextlib.ExitStack() as ph1:
            wkvp = ph1.enter_context(tc.tile_pool(name="wkvp", bufs=1))
            wkv_sb = wkvp.tile([128, KT, 2 * C], bf16, name="wkv_sb")
            for kt in range(KT):
                nc.sync.dma_start(wkv_sb[:, kt, :], P["wqkv"][:, kt, C:3 * C])
            for kt in range(KT):
                nc.sync.dma_start(wq_sb[:, kt, :], P["wqkv"][:, kt, 0:C])
            for kt in range(KT):
                nc.sync.dma_start(wo_sb[:, kt, :], P["wo"][:, kt, :])

            gpp = ph1.enter_context(tc.tile_pool(name="gpp", bufs=1, space="PSUM"))

            # ---- gate: z^T = Wg x^T  [16, QT]
            gp = gpp.tile([16, QT], f32, name="gp")
            for kt in range(KT):
                nc.tensor.matmul(gp[:], wgT_sb[:, kt, :], xtb_sb[:, kt, :],
                                 start=(kt == 0), stop=(kt == KT - 1))
            gs = wkvp.tile([16, QT], f32, name="gs")
            nc.vector.tensor_scalar(gs[:], gp[:], bg_sb[:, 0:1], None, ALU.add)
            nc.scalar.activation(gs[:], gs[:], AF.Sigmoid)
            nc.vector.tensor_scalar(gs[:], gs[:], 0.999, 0.01, ALU.min, ALU.max)
            if has_mask:
                # f' = (f-1)*mk + 1
                nc.vector.scalar_tensor_tensor(gs[:], gs[:], 1.0, mkT_sb[:],
                                               op0=ALU.subtract, op1=ALU.mult)
                nc.vector.tensor_scalar_add(gs[:], gs[:], 1.0)
            nc.scalar.activation(gs[:], gs[:], AF.Ln)
            gcs = wkvp.tile([16, QT], f32, name="gcs")      # cumsum log f
            nc.vector.tensor_tensor_scan(gcs[:], gs[:], gs[:], 0.0,
                                         op0=ALU.add, op1=ALU.bypass)
            nc.vector.tensor_scalar_max(gcs[:], gcs[:], G_CLAMP)

            # exp tables (bf16), transpose per chunk -> row-major b, 1/b
            bT = wkvp.tile([16, QT], bf16, name="bT")
            biT = wkvp.tile([16, QT], bf16, name="biT")
            nc.scalar.activation(bT[:], gcs[:], AF.Exp)
            nc.scalar.activation(biT[:], gcs[:], AF.Exp, scale=-1.0)
            eyeb = eye_sb[0:16, 0:16]
            for ch in range(NCH):
                for src_t, dst in ((bT, b_rm), (biT, bi_rm)):
                    gtp = gpp.tile([128, 16], bf16, name="gtp", tag="gsm")
                    nc.tensor.matmul(gtp[:], src_t[:, bass.ts(ch, L)], eyeb,
                                     is_transpose=True, start=True, stop=True)
                    nc.vector.tensor_copy(dst[:, ch, :], gtp[:])
            if has_mask:
                for ch in range(NCH):
                    nc.vector.tensor_mul(
                        b_rm[:, ch, :], b_rm[:, ch, :],
                        mkc_sb[:, ch:ch + 1].to_broadcast([128, H]))
                    nc.vector.tensor_mul(
                        bi_rm[:, ch, :], bi_rm[:, ch, :],
                        mkc_sb[:, ch:ch + 1].to_broadcast([128, H]))

            # A (end-of-quarter decay) broadcast to 64 partitions
            gta = gpp.tile([1, 16], bf16, name="gta", tag="gsm")
            nc.tensor.matmul(gta[:], bT[:, QT - 1:QT], eyeb,
                             is_transpose=True, start=True, stop=True)
            arow = keep.tile([1, 16], bf16, name="arow")
            nc.vector.tensor_copy(arow[:], gta[:])
            abc = gpp.tile([64, 16], f32, name="abc", tag="gsm")
            nc.tensor.matmul(abc[:], triu_sb[0:1, 0:64], arow[:],
                             start=True, stop=True)
            nc.vector.tensor_copy(arun[:], abc[:])

            ppk = ph1.enter_context(tc.tile_pool(name="ppk", bufs=2, space="PSUM"))
            tp = ph1.enter_context(tc.tile_pool(name="tp", bufs=1, space="PSUM"))
            up = ph1.enter_context(tc.tile_pool(name="up", bufs=1, space="PSUM"))

            def kv_proj(ch):
                tsl = bass.ts(ch, L)
                kp = ppk.tile([128, C], f32, name="kp", tag="pk")
                vp = ppk.tile([128, C], f32, name="vp", tag="pk")
                for kt in range(KT):
                    for nh in range(2):
                        nc.tensor.matmul(kp[:, bass.ts(nh, 512)],
                                         xtb_sb[:, kt, tsl],
                                         wkv_sb[:, kt, nh * 512:(nh + 1) * 512],
                                         start=(kt == 0), stop=(kt == KT - 1))
                    for nh in range(2):
                        nc.tensor.matmul(vp[:, bass.ts(nh, 512)],
                                         xtb_sb[:, kt, tsl],
                                         wkv_sb[:, kt, C + nh * 512:C + (nh + 1) * 512],
                                         start=(kt == 0), stop=(kt == KT - 1))
                return kp, vp

            def k_side(ch, kp, vp):
                ktm = rope_phi(ch, kp, "k")
                kt_c = ktp.tile([64, H, 128], bf16, name=f"kt{ch}", tag="kt")
                transpose_to(kt_c, ktm, tp)
                kt_tiles.append(kt_c)
                # vext
                vext = vxp.tile([128, H, SE], bf16, name=f"vx{ch}", tag="vx")
                if has_mask:
                    nc.scalar.activation(
                        vext[:, :, 0:D], vp.rearrange("p (h d) -> p h d", h=H),
                        AF.Copy, scale=mkc_sb[:, ch:ch + 1])
                else:
                    nc.scalar.copy(vext[:, :, 0:D],
                                   vp.rearrange("p (h d) -> p h d", h=H))
                nc.gpsimd.tensor_copy(vext[:, :, D], one_b.to_broadcast([128, H]))
                vx_tiles.append(vext)
                # ΔU accumulate + bf16 snapshot
                u_c = usp.tile([64, H, SE], f32, name="u_c", tag="u_c")
                for g in range(4):
                    ups = up.tile([64, 4, SE], f32, name="ups", tag="ups")
                    for j in range(4):
                        h = g * 4 + j
                        nc.tensor.matmul(ups[:, j, :], ktm[:, h, :],
                                         vext[:, h, :],
                                         start=(j == 0), stop=(j == 3))
                    sl = slice(g * 4, (g + 1) * 4)
                    if ch == 0:
                        nc.vector.tensor_copy(u_c[:, sl, :], ups[:])
                    else:
                        nc.vector.tensor_add(u_c[:, sl, :], ups[:],
                                             u_tiles[ch - 1][:, sl, :])
                u_tiles.append(u_c)


            for ch in range(NCH):
                kp, vp = kv_proj(ch)
                k_side(ch, kp, vp)

            # ---- exchange: seff = A * U_full
            seff = keep.tile([64, H, SE], f32, name="seff")
            nc.vector.tensor_mul(
                seff[:], u_tiles[-1][:],
                arun[:, :, None].to_broadcast([64, H, SE]))

        cc_in = dram.tile([64, H * SE + H], f32, name="cc_in")
        cc_out = dram.tile([256, H * SE + H], f32, name="cc_out")
        nc.sync.dma_start(cc_in[:, 0:H * SE], seff.rearrange("p h e -> p (h e)"))
        nc.sync.dma_start(cc_in[:, H * SE:], arun[:])
        nc.gpsimd.collective_compute(
            "AllGather", ALU.bypass,
            replica_groups=[[0, 1, 2, 3], [4, 5, 6, 7]],
            ins=[cc_in.opt()], outs=[cc_out.opt()])

        # ================================================ scope 2: q side
        with contextlib.ExitStack() as ph2:
            ppq = ph2.enter_context(tc.tile_pool(name="ppq", bufs=1, space="PSUM"))
            atp2 = ph2.enter_context(tc.tile_pool(name="atp2", bufs=2, space="PSUM"))
            ipp = ph2.enter_context(tc.tile_pool(name="ipp", bufs=2, space="PSUM"))

            def q_proj(ch):
                tsl = bass.ts(ch, L)
                qp = ppq.tile([128, C], f32, name="qp", tag="pq")
                for kt in range(KT):
                    for nh in range(2):
                        nc.tensor.matmul(qp[:, bass.ts(nh, 512)],
                                         xtb_sb[:, kt, tsl],
                                         wq_sb[:, kt, bass.ts(nh, 512)],
                                         start=(kt == 0), stop=(kt == KT - 1))
                return qp

            def at_mm(ch, qt_c, g, ats):
                for j in range(4):
                    h = g * 4 + j
                    nc.tensor.matmul(ats[:, j, :], kt_tiles[ch][:, h, :],
                                     qt_c[:, h, :],
                                     start=(j == 0), stop=(j == 3))

            def q_side(ch, qp):
                qrow = rope_phi(ch, qp, "q")
                qt_c = qtp.tile([64, H, 128], bf16, name=f"qt{ch}", tag="qt")
                transpose_to(qt_c, qrow, atp2)
                qt_tiles.append(qt_c)
                # intra-chunk attention + q.U_prev into one PSUM accumulation
                tn = tnp.tile([128, H, SE], f32, name=f"tn{ch}", tag="tn")
                ats_list = [atp2.tile([128, 4, 128], f32, name="ats", tag="ats")]
                at_mm(ch, qt_c, 0, ats_list[0])
                for g in range(4):
                    ats = ats_list[g]
                    atm = sm.tile([128, 4, 128], bf16, name="atm", tag="atm")
                    nc.vector.tensor_mul(
                        atm[:], ats[:],
                        triu_sb[:, None, :].to_broadcast([128, 4, 128]))
                    if g + 1 < 4:
                        nats = atp2.tile([128, 4, 128], f32, name="ats", tag="ats")
                        at_mm(ch, qt_c, g + 1, nats)
                        ats_list.append(nats)
                    ips = ipp.tile([128, 4, SE], f32, name="ips", tag="ips")
                    for j in range(4):
                        h = g * 4 + j
                        nc.tensor.matmul(ips[:, j, :], atm[:, j, :],
                                         vx_tiles[ch][:, h, :],
                                         start=(j == 0), stop=(j == 3))
                    sl = slice(g * 4, (g + 1) * 4)
                    nc.scalar.copy(tn[:, sl, :], ips[:])
                tn_tiles.append(tn)

            for ch in range(NCH):
                q_side(ch, q_proj(ch))

            # ---- combine peers -> sstart (fp32), split across DVE/Pool
            sstart_b = keep.tile([64, H, SE], f32, name="sstart_b")
            peers, aqs = [], []
            for q in range(4):
                peer = sm.tile([64, H * SE + H], f32, name=f"peer{q}",
                               tag=f"peer{q % 2}")
                nc.sync.dma_start(peer[:], cc_out[q * 64:(q + 1) * 64, :])
                peers.append(peer)
                if q == 0:
                    aqs.append(None)
                    continue
                aq = keep.tile([64, H], f32, name=f"aq{q}")
                nc.vector.scalar_tensor_tensor(
                    aq[:], peer[:, H * SE:], sel_sb[:, q:q + 1],
                    isel_sb[:, q:q + 1].to_broadcast([64, H]),
                    op0=ALU.mult, op1=ALU.add)
                aqs.append(aq)
            acc = seff  # dead after cc_in DMA; reuse as combine accumulator
            for half in range(2):
                hs = slice(half * 8, (half + 1) * 8)
                fs = slice(half * 8 * SE, (half + 1) * 8 * SE)
                eng = nc.vector if half == 0 else nc.gpsimd
                nhf = 8 * SE
                accv = acc.rearrange("p h e -> p (h e)")[:, fs]
                pv0 = peers[0][:, 0:H * SE][:, fs]
                eng.tensor_mul(accv, pv0,
                               sel_sb[:, 0:1].to_broadcast([64, nhf]))
                for q in range(1, 4):
                    eng.tensor_mul(
                        acc[:, hs, :], acc[:, hs, :],
                        aqs[q][:, hs, None].to_broadcast([64, 8, SE]))
                    pvq = peers[q][:, 0:H * SE][:, fs]
                    dst = sstart_b if q == 3 else acc
                    dv = dst.rearrange("p h e -> p (h e)")[:, fs]
                    if half == 0:
                        eng.scalar_tensor_tensor(
                            dv, pvq, sel_sb[:, q:q + 1], accv,
                            op0=ALU.mult, op1=ALU.add)
                    else:
                        tmpv = sm.tile([64, 8, SE], f32, name="cmb",
                                       tag="cmb").rearrange("p h e -> p (h e)")
                        eng.tensor_mul(tmpv, pvq,
                                       sel_sb[:, q:q + 1].to_broadcast([64, nhf]))
                        eng.tensor_add(dv, tmpv, accv)

            # per-chunk effective state entering the chunk (bf16)
            for ch in range(NCH):
                sacb = sm.tile([64, H, SE], bf16, name=f"sacb{ch}", tag="sacb", bufs=NCH)
                if ch == 0:
                    eng = nc.gpsimd
                    eng.tensor_copy(sacb[:], sstart_b[:])
                else:
                    eng = nc.vector if ch % 2 == 0 else nc.gpsimd
                    eng.tensor_add(sacb[:], sstart_b[:], u_tiles[ch - 1][:])
                sacb_tiles.append(sacb)

            # ---- finalize chunks: q.(S_start+U_prev), normalize, o-proj, LN
            xrp = ph2.enter_context(tc.tile_pool(name="xrp", bufs=1))
            ysp = ph2.enter_context(tc.tile_pool(name="ysp", bufs=2))

            for ch in range(NCH):
                tn = tn_tiles[ch]
                for g in range(4):
                    ip2 = ipp.tile([128, 4, SE], f32, name="ip2", tag="ips")
                    for j in range(4):
                        h = g * 4 + j
                        nc.tensor.matmul(ip2[:, j, :], qt_tiles[ch][:, h, :],
                                         sacb_tiles[ch][:, h, :],
                                         start=(j == 0), stop=(j == 3))
                    sl = slice(g * 4, (g + 1) * 4)
                    nc.vector.tensor_add(tn[:, sl, :], tn[:, sl, :], ip2[:])
                den = sm.tile([128, H], f32, name="den", tag="den", bufs=1)
                nc.vector.tensor_scalar_max(den[:], tn[:, :, D], EPS)
                nc.vector.reciprocal(den[:], den[:])
                attn = sm.tile([128, H, D], bf16, name="attn", tag="attn", bufs=1)
                nc.vector.tensor_mul(attn[:], tn[:, :, 0:D],
                                     den[:, :, None].to_broadcast([128, H, D]))
                # transpose attn -> C-major, o-proj, residual, LN
                at_sb = sm.tile([128, KT, 128], bf16, name="at_sb", tag="at_sb", bufs=1)
                af = attn.rearrange("p h d -> p (h d)")
                for tg in range(2):
                    tps = atp2.tile([128, 4, 128], bf16, name="tps2", tag="tps")
                    for j in range(4):
                        nc.tensor.matmul(tps[:, j, :],
                                         af[:, bass.ts(tg * 4 + j, 128)],
                                         eye_sb[:], is_transpose=True,
                                         start=(j == 0), stop=(j == 3))
                    sl = slice(tg * 4, (tg + 1) * 4)
                    nc.scalar.copy(at_sb[:, sl, :], tps[:])

                xr = xrp.tile([128, C], f32, name="xr", tag="xr")
                nc.sync.dma_start(xr[:], P["xrows"][bass.ts(ch, 128), :])
                ysb = xrp.tile([128, C], f32, name="ysb", tag="ysb")
                mus = sm.tile([128, 8], f32, name="mus", tag="mus", bufs=1)
                ops = ppq.tile([128, C], f32, name="ops", tag="pq")
                for kt in range(KT):
                    for nh in range(2):
                        nsl = bass.ts(nh, 512)
                        nc.tensor.matmul(ops[:, nsl], at_sb[:, kt, :],
                                         wo_sb[:, kt, nsl],
                                         start=(kt == 0), stop=(kt == KT - 1))
                nc.vector.tensor_add(ysb[:], ops[:], xr[:])
                yln = ysp.tile([128, C], f32, name="yln", tag="yln")
                nc.scalar.activation(yln[:], ysb[:], AF.Identity,
                                     accum_out=mus[:, 2:3])
                nc.vector.tensor_scalar_mul(mus[:, 3:4], mus[:, 2:3], -1.0 / C)
                nc.scalar.activation(yln[:], ysb[:], AF.Square,
                                     bias=mus[:, 3:4], accum_out=mus[:, 4:5])
                nc.vector.tensor_scalar(mus[:, 4:5], mus[:, 4:5], 1.0 / C,
                                        LN_EPS, ALU.mult, ALU.add)
                nc.scalar.activation(mus[:, 4:5], mus[:, 4:5], AF.Sqrt)
                nc.vector.reciprocal(mus[:, 4:5], mus[:, 4:5])
                nc.vector.tensor_mul(mus[:, 5:6], mus[:, 3:4], mus[:, 4:5])
                nc.scalar.activation(yln[:], ysb[:], AF.Identity,
                                     scale=mus[:, 4:5], bias=mus[:, 5:6])
                if has_ln:
                    nc.vector.tensor_mul(yln[:], yln[:], lnw_sb[:])
                    nc.vector.tensor_add(yln[:], yln[:], lnb_sb[:])
                nc.sync.dma_start(P["y"][bass.ts(ch, 128), :], yln[:])

    from concourse.library_overlay import lower_extended_insts
    lower_extended_insts(nc)
    if SPLIT_WAITS:
        _split_multi_waits(nc)
    return nc


# ---------------------------------------------------------------- host side
def _rope_tables():
    half = D // 2
    inv = 1.0 / (ROPE_BASE ** (np.arange(half, dtype=np.float64) / half))
    t = np.arange(T, dtype=np.float64)
    fr = t[:, None] * inv[None, :]
    cos, sin = np.cos(fr), np.sin(fr)
    out = np.zeros((T, 128), np.float32)
    out[:, 0:32] = cos
    out[:, 32:64] = cos
    out[:, 64:96] = -sin
    out[:, 96:128] = sin
    return out


def _ktile(w, dtype=np.float32):  # [C, N] -> [128, KT, N]
    return np.ascontiguousarray(
        w.reshape(KT, 128, w.shape[1]).transpose(1, 0, 2)).astype(dtype)


_cache = {}
RUN_KW = {}      # extra kwargs for run_bass_kernel_spmd (test harness profiling)
LAST = None      # last BassKernelResults (test harness reads exec_time_ns)


def kernel(x, mask, Wq, Wk, Wv, Wg, bg, Wo, bo, ln_w, ln_b):
    bfl = ml_dtypes.bfloat16
    x = np.asarray(x, np.float32)
    mask = np.asarray(mask)
    has_mask = not np.all(mask == 1)
    has_ln = not (np.all(np.asarray(ln_w) == 1) and np.all(np.asarray(ln_b) == 0))

    key = (has_mask, has_ln)
    if key not in _cache:
        _cache[key] = build(has_mask, has_ln)
    nc = _cache[key]

    wqkv = _ktile(np.concatenate(
        [np.asarray(Wq).T, np.asarray(Wk).T, np.asarray(Wv).T], axis=1), bfl)
    wgt = _ktile(np.ascontiguousarray(np.asarray(Wg, np.float32).T), bfl)
    wo_t = _ktile(np.ascontiguousarray(np.asarray(Wo).T), bfl)
    ropec_full = _rope_tables()
    triu = np.triu(np.ones((128, 128), np.float32))
    eye = np.eye(128)
    onesrow = np.ones((1, 128), np.float32)
    bgcol = np.asarray(bg, np.float32)[:, None]
    bo_f = np.asarray(bo, np.float32)

    in_maps = []
    for c in range(NCORE):
        b, r = c // 4, c % 4
        rows = slice(r * QT, (r + 1) * QT)
        xq = np.ascontiguousarray(x[b, rows].T)   # [C, QT]
        m = {
            "xTb": _ktile(xq, bfl),
            "wqkv": wqkv,
            "wgT": wgt,
            "wo": wo_t,
            "xrows": np.ascontiguousarray(x[b, rows]) + bo_f[None, :],
            "ropec": np.ascontiguousarray(
                ropec_full[rows].reshape(NCH, 128, 128).transpose(1, 0, 2)
            ).astype(bfl),
            "triu": triu.astype(bfl),
            "eye": eye.astype(bfl),
            "eyef": eye[:16, :16].astype(np.float32),
            "onesrow": onesrow,
            "bgcol": bgcol,
        }
        sel = np.zeros((64, 4), np.float32)
        sel[:, 0:r] = 1.0
        m["sel"] = sel
        m["isel"] = 1.0 - sel
        if has_mask:
            mk = np.asarray(mask[b, rows], np.float32)
            m["mkc"] = np.ascontiguousarray(mk.reshape(NCH, 128).T)
            m["mkT"] = np.tile(mk[None, :], (16, 1))
        if has_ln:
            m["lnw"] = np.tile(np.asarray(ln_w, np.float32), (128, 1))
            m["lnb"] = np.tile(np.asarray(ln_b, np.float32), (128, 1))
        in_maps.append(m)

    res = run_bass_kernel_spmd(nc, in_maps, list(range(NCORE)), **RUN_KW)
    globals()["LAST"] = res
    out = np.empty((B, T, C), np.float32)
    for c in range(NCORE):
        b, r = c // 4, c % 4
        out[b, r * QT:(r + 1) * QT, :] = res.results[c]["y"]
    return out
